# revision 28
# baseline (speedup 1.0000x reference)
# MultiLoraConv2d kernel for 8 trn2 NeuronCores (Bass/Tile, data-parallel over batch).
#
# Math (per sample b):
#   delta_flat[b] = sum_t 2*alphas[b,t] * (lora_B[t] @ lora_A[t])        [768, 768]
#   agg[b] = W + delta_flat[b].reshape(COUT, CIN, 3, 3)                  (flat reinterpret)
#   out[b] = conv2d(x[b], agg[b], pad=1)
#
# Device strategy (per core, S = B/8 samples):
#   - Phase 1 (unchanged from direct-conv version): per-sample aggregated
#     conv weights come out of the PE in c-major layout via LoRA matmuls;
#     evictions land in w-tap slots of u1 tiles, then 3 vector ops per
#     (sample, cin-tile) build the 1D-Winograd F(2,3) weight transform
#       m0 = g0, m1 = g0+g1+g2, m2 = g0-g1+g2, m3 = g2  (unscaled; the
#     1/2 is folded into the output combine).
#   - Phase 2: conv via 1D Winograd along W. Input transform (4 tensor
#     ops per x quarter, bf16, host-deinterleaved even/odd columns):
#       V[0]=E_t-E_{t+1}, V[1]=O_t+E_{t+1}, V[2]=E_{t+1}-O_t, V[3]=O_t-O_{t+1}
#     GEMM per 16-row strip: M[m] = sum_{i,ct} u1[.,i,m,.]^T V[.,h+i,m,.]
#     (24 matmuls of free 512, bf16, 4 PSUM banks), then output combine
#       y_even = M0 + (M1+M2)/2,  y_odd = (M1-M2)/2 - M3
#     on vector/gpsimd, writing the interleaved output tile directly.
#   PE work drops to 2/3 of direct conv (6 instead of 9 multiplies per
#   output pair along the W axis).
import numpy as np

B, T, R, ALPHA = 32, 4, 8, 16
CIN, COUT, K = 256, 256, 3
H = W_SP = 64
SCALING = ALPHA / R
NCORES = 8
S = B // NCORES      # samples per core
NR = T * R * K       # 96 lora rows (padded to 128 partitions)
P = 128
HP = H + 2           # 66 padded
HH = 34              # padded-row half-tile height (rows 0:34 and 32:66)

_CACHE = {}


def _build_nc():
    import concourse.bacc as bacc
    import concourse.mybir as mybir
    import concourse.tile as tile

    f32 = mybir.dt.float32
    f32r = mybir.dt.float32r
    bf16 = mybir.dt.bfloat16
    ALU = mybir.AluOpType

    nc = bacc.Bacc("TRN2", target_bir_lowering=False, debug=False, num_devices=NCORES)

    # x padded + column-deinterleaved (even/odd) in bf16
    xq = nc.declare_dram_parameter("xp", [S, 2, P, HP, 2, 33], bf16, isOutput=False)
    # base weights and LoRA A factor pre-transformed by the Winograd G
    # matrix on host: the (i,m) index replaces the 9 conv taps.
    wt = nc.declare_dram_parameter("wt", [P, 3, 4, 2, COUT], bf16, isOutput=False)
    a3 = nc.declare_dram_parameter("a3", [P, 12, 3, CIN], bf16, isOutput=False)
    b3 = nc.declare_dram_parameter("b3", [P, 3, COUT], f32, isOutput=False)
    alph = nc.declare_dram_parameter("alph", [P, S], f32, isOutput=False)
    outd = nc.declare_dram_parameter("out", [S, 2, P, H, W_SP], f32, isOutput=True)

    HSMP = S // 2  # samples per 512-wide delta-matmul half

    with tile.TileContext(nc) as tc:
        with tc.tile_pool(name="u1_pool", bufs=S) as u1_pool, \
             tc.tile_pool(name="xq_pool", bufs=4) as xq_pool, \
             tc.tile_pool(name="v1_pool", bufs=5) as v1_pool:
            # per-sample Winograd-transformed weights, c-major:
            # u1[c, ct, i, m, o] bf16
            u1s = [u1_pool.tile([P, 2, 3, 4, COUT], bf16, name="u1")
                   for _ in range(S)]
            # V1 quarters: [c, 34 padded rows, m, tj] bf16
            v1q = {}   # (smp, ct, half) -> tile, kept for current+next stage
            xqt = {}

            def issue_xq(smp, ct, half):
                t = xq_pool.tile([P, HH, 2, 33], bf16, name="xq")
                nc.sync.dma_start(
                    t[:, :, :, :], xq[smp, ct, :, 32 * half:32 * half + HH, :, :])
                xqt[(smp, ct, half)] = t

            def transform(smp, ct, half, split=False):
                # gpsimd cannot read PSUM, so it gets the (all-SBUF) input
                # transforms while the vector engine owns the PSUM-reading
                # output combines (keeping them off each other's queues).
                # split=True spreads the 4 ops across both engines (used at
                # stage 0 where transform latency gates the conv start).
                xt = xqt.pop((smp, ct, half))
                v = v1_pool.tile([P, HH, 4, 32], bf16, name="v1")
                E0 = xt[:, :, 0, 0:32]
                E1 = xt[:, :, 0, 1:33]
                O0 = xt[:, :, 1, 0:32]
                O1 = xt[:, :, 1, 1:33]
                e0 = nc.vector if split else nc.gpsimd
                e0.tensor_sub(v[:, :, 0, :], E0, E1)
                e0.tensor_add(v[:, :, 1, :], O0, E1)
                nc.gpsimd.tensor_sub(v[:, :, 2, :], E1, O0)
                nc.gpsimd.tensor_sub(v[:, :, 3, :], O0, O1)
                v1q[(smp, ct, half)] = v

            # ---- phase 1: Winograd-transformed weights via LoRA matmuls ----
            # a3/wt carry the G transform from host, so the GEMM evictions
            # land directly in the final u1 (i,m) slots. For each ct block
            # only s in {ct, ct+1} rows of a3 are nonzero (the c*9+d flat-
            # index decomposition), so the third s-matmul is skipped.
            with tc.tile_pool(name="cst", bufs=1) as cst, \
                 tc.tile_pool(name="dps", bufs=2, space="PSUM") as dps:
                a3_sb = cst.tile([P, 12, 3, CIN], bf16)
                b3_sb = cst.tile([P, 3, COUT], f32)
                alph_sb = cst.tile([P, S], f32)
                wt_sb = cst.tile([P, 3, 4, 2, COUT], bf16)
                b3s_lo = cst.tile([P, 3, S // 2, COUT], bf16)
                b3s_hi = cst.tile([P, 3, S // 2, COUT], bf16)

                # DMA completes strictly in issue order. The delta GEMM
                # consumes a3[dm] every ~2us and the evictions need wt[dm]
                # shortly after, so issue those in fine-grained alternating
                # chunks; the sample-0 x quarters follow.
                nc.sync.dma_start(b3_sb[:, :, :], b3[:, :, :])
                nc.sync.dma_start(alph_sb[:, :], alph[:, :])
                nc.sync.dma_start(a3_sb[:, 0:2], a3[:, 0:2])
                nc.sync.dma_start(wt_sb[:, 0:1], wt[:, 0:1])
                nc.sync.dma_start(a3_sb[:, 2:6], a3[:, 2:6])
                nc.sync.dma_start(wt_sb[:, 1:2], wt[:, 1:2])
                nc.sync.dma_start(a3_sb[:, 6:12], a3[:, 6:12])
                nc.sync.dma_start(wt_sb[:, 2:3], wt[:, 2:3])
                issue_xq(0, 0, 0)
                issue_xq(0, 1, 0)
                issue_xq(0, 0, 1)
                issue_xq(0, 1, 1)

                # alphas*scaling on vector; the per-sample b3 scaling runs
                # on the scalar engine (activation Copy with per-partition
                # scale AP), keeping the vector queue free for evictions.
                nc.vector.tensor_scalar_mul(alph_sb[:, :], alph_sb[:, :],
                                            float(SCALING))
                Copy = mybir.ActivationFunctionType.Copy
                for smp in range(S):
                    dst = b3s_lo if smp < HSMP else b3s_hi
                    for s in range(3):
                        nc.scalar.activation(
                            dst[:, s, smp % HSMP, :], b3_sb[:, s, :], Copy,
                            bias=0.0, scale=alph_sb[:, smp:smp + 1])

                for dm in range(12):
                    i, m = divmod(dm, 4)
                    # dp holds both ct blocks: [ct, half, hsmp, o]
                    dp = dps.tile([P, 2, 2, HSMP, COUT], f32, name="dp")
                    for ct in range(2):
                        for half in range(2):
                            for s in (ct, ct + 1):
                                nc.tensor.matmul(
                                    dp[:, ct, half, :, :],
                                    a3_sb[:, dm, s, ct * P:(ct + 1) * P],
                                    (b3s_lo if half == 0 else b3s_hi)[:, s, :, :],
                                    start=(s == ct), stop=(s == ct + 1))
                    # one eviction per sample covering both ct blocks
                    for smp in range(S):
                        nc.vector.tensor_add(
                            u1s[smp][:, :, i, m, :],
                            dp[:, :, smp // HSMP, smp % HSMP, :],
                            wt_sb[:, i, m, :, :])

            # stage-0 input transforms, emitted before phase 2 so they sit
            # at the front of the (otherwise empty) engine queues
            transform(0, 0, 0, split=True)
            transform(0, 1, 0, split=True)

            # ---- phase 2: 1D-Winograd conv ----
            with tc.tile_pool(name="ob_pool", bufs=4) as ob_pool, \
                 tc.tile_pool(name="tt_pool", bufs=6) as tt_pool, \
                 tc.tile_pool(name="cps", bufs=2, space="PSUM") as cps:
                stages = [(smp, half) for smp in range(S) for half in range(2)]
                strip_no = 0
                for k, (smp, half) in enumerate(stages):
                    # prefetch + transform the next stage's quarters
                    if k + 1 < len(stages):
                        nsmp, nhalf = stages[k + 1]
                        if nsmp != 0:
                            issue_xq(nsmp, 0, nhalf)
                            issue_xq(nsmp, 1, nhalf)
                        transform(nsmp, 0, nhalf)
                        transform(nsmp, 1, nhalf)
                    va = v1q.pop((smp, 0, half))
                    vb = v1q.pop((smp, 1, half))
                    for ot in range(2):
                        # the very last strip is split in two so the final
                        # combine+DMA tail after the last matmul is shorter
                        strips = [(0, 16), (16, 16)]
                        if k == len(stages) - 1 and ot == 1:
                            strips = [(0, 16), (16, 8), (24, 8)]
                        for r0, rows in strips:
                            pst = cps.tile([P, 4, rows, 32], f32, name="pst")
                            for m in range(4):
                                for i in range(3):
                                    for ct in range(2):
                                        nc.tensor.matmul(
                                            pst[:, m, :, :],
                                            u1s[smp][:, ct, i, m,
                                                     ot * P:(ot + 1) * P],
                                            (va if ct == 0 else vb)[
                                                :, r0 + i:r0 + i + rows, m, :],
                                            start=(i == 0 and ct == 0),
                                            stop=(i == 2 and ct == 1))
                            # output combine:
                            #   even = M0 + (M1+M2)/2 ; odd = (M1-M2)/2 - M3
                            # Engine ops may read only ONE psum operand, so
                            # the scalar engine first evacuates M1 to SBUF.
                            eng = nc.vector
                            strip_no += 1
                            ob = ob_pool.tile([P, rows, 32, 2], f32, name="ob")
                            s1 = tt_pool.tile([P, rows, 32], f32, name="s1")
                            t1 = tt_pool.tile([P, rows, 32], f32, name="tt")
                            t2 = tt_pool.tile([P, rows, 32], f32, name="tt")
                            nc.scalar.copy(s1[:, :, :], pst[:, 1, :, :])
                            eng.tensor_add(t1[:, :, :], pst[:, 2, :, :],
                                           s1[:, :, :])
                            eng.scalar_tensor_tensor(
                                ob[:, :, :, 0], t1[:, :, :], 0.5,
                                pst[:, 0, :, :], ALU.mult, ALU.add)
                            eng.scalar_tensor_tensor(
                                t2[:, :, :], pst[:, 2, :, :], -1.0,
                                s1[:, :, :], ALU.mult, ALU.add)
                            eng.scalar_tensor_tensor(
                                ob[:, :, :, 1], t2[:, :, :], 0.5,
                                pst[:, 3, :, :], ALU.mult, ALU.subtract)
                            nc.sync.dma_start(
                                outd[smp, ot, :,
                                     32 * half + r0:32 * half + r0 + rows, :],
                                ob[:, :, :, :])
    nc.finalize()
    return nc


def _host_prep(x, alphas, W, lora_A, lora_B):
    """Host-side layout transforms (pad/transpose/gather/deinterleave)."""
    import ml_dtypes
    bf16 = ml_dtypes.bfloat16

    xf = np.ascontiguousarray(np.asarray(x, dtype=np.float32))
    af = np.asarray(alphas, dtype=np.float32)
    Wf = np.asarray(W, dtype=np.float32)
    Af = np.asarray(lora_A, dtype=np.float32).reshape(NR, CIN * K)   # Acat
    Bf = np.asarray(lora_B, dtype=np.float32)

    # padded x, bf16, even/odd columns deinterleaved:
    # (S, 2, 128, 66, 2, 33) per core
    xpad = np.zeros((B, CIN, HP, HP), np.float32)
    xpad[:, :, 1:-1, 1:-1] = xf
    xde = np.empty((B, CIN, HP, 2, 33), bf16)
    xde[:, :, :, 0, :] = xpad[:, :, :, 0::2]
    xde[:, :, :, 1, :] = xpad[:, :, :, 1::2]
    xde = xde.reshape(NCORES, S, 2, P, HP, 2, 33)

    # Winograd F(2,3) weight transform over the w-taps (unscaled G; the
    # 1/2 is folded into the device-side output combine)
    G = np.array([[1, 0, 0], [1, 1, 1], [1, -1, 1], [0, 0, 1]], np.float32)

    # base weights c-major: wt[p, i, m, ct, o] = G-combo of [p, (i,j), ct, o]
    wt9 = (Wf.reshape(COUT, CIN, 9).transpose(1, 2, 0)     # [c, d, o]
           .reshape(2, P, 3, 3, COUT)                      # [ct, p, i, j, o]
           .transpose(1, 2, 3, 0, 4))                      # [p, i, j, ct, o]
    wth = np.ascontiguousarray(
        np.einsum('mj,pijto->pimto', G, wt9)).astype(bf16)

    # a3[r, d, s, c] = Acat[r, c*9+d-768*s] masked; rows padded 96 -> 128,
    # then G-folded over the j tap: a3m[r, (i,m), s, c]
    a3h = np.zeros((P, 9, 3, CIN), np.float32)
    cc = np.arange(CIN)
    for d in range(9):
        q = cc * 9 + d
        s_of_c = q // (CIN * K)
        q_of_c = q % (CIN * K)
        for s in range(3):
            m = s_of_c == s
            a3h[:NR, d, s, m] = Af[:, q_of_c[m]]
    a3h = np.ascontiguousarray(
        np.einsum('mj,rijsc->rimsc', G, a3h.reshape(P, 3, 3, 3, CIN))
        .reshape(P, 12, 3, CIN)).astype(bf16)

    # b3[r, s, o] = Bcat[3o+s, r];  Bcat = lora_B transposed to [768, 96]
    Bcat = Bf.transpose(1, 0, 2).reshape(COUT * K, NR)
    b3h = np.zeros((P, 3, COUT), np.float32)
    b3h[:NR] = Bcat.reshape(COUT, 3, NR).transpose(2, 1, 0)

    # alph[r, smp] per core (repeat each task 24x; zero rows >= 96)
    alphh = np.zeros((NCORES, P, S), np.float32)
    rep = np.repeat(af, R * K, axis=1)                     # [B, 96]
    alphh[:, :NR, :] = rep.reshape(NCORES, S, NR).transpose(0, 2, 1)

    return xde, wth, a3h, b3h, alphh


def kernel(x, alphas, W, lora_A, lora_B):
    from concourse.bass_utils import run_bass_kernel_spmd

    if "nc" not in _CACHE:
        _CACHE["nc"] = _build_nc()
    nc = _CACHE["nc"]

    xpad, wth, a3h, b3h, alphh = _host_prep(x, alphas, W, lora_A, lora_B)
    in_maps = [
        {"xp": np.ascontiguousarray(xpad[c]), "wt": wth, "a3": a3h, "b3": b3h,
         "alph": np.ascontiguousarray(alphh[c])}
        for c in range(NCORES)
    ]
    res = run_bass_kernel_spmd(nc, in_maps, list(range(NCORES)))
    out = np.empty((B, COUT, H, W_SP), np.float32)
    for c in range(NCORES):
        out[c * S:(c + 1) * S] = res.results[c]["out"].reshape(S, COUT, H, W_SP)
    return out


# revision 31
# speedup vs baseline: 1.0560x; 1.0560x over previous
# MultiLoraConv2d kernel for 8 trn2 NeuronCores (Bass/Tile, data-parallel over batch).
#
# Math (per sample b):
#   delta_flat[b] = sum_t 2*alphas[b,t] * (lora_B[t] @ lora_A[t])        [768, 768]
#   agg[b] = W + delta_flat[b].reshape(COUT, CIN, 3, 3)                  (flat reinterpret)
#   out[b] = conv2d(x[b], agg[b], pad=1)
#
# Device strategy (per core, S = B/8 samples):
#   - Phase 1 (unchanged from direct-conv version): per-sample aggregated
#     conv weights come out of the PE in c-major layout via LoRA matmuls;
#     evictions land in w-tap slots of u1 tiles, then 3 vector ops per
#     (sample, cin-tile) build the 1D-Winograd F(2,3) weight transform
#       m0 = g0, m1 = g0+g1+g2, m2 = g0-g1+g2, m3 = g2  (unscaled; the
#     1/2 is folded into the output combine).
#   - Phase 2: conv via 1D Winograd along W. Input transform (4 tensor
#     ops per x quarter, bf16, host-deinterleaved even/odd columns):
#       V[0]=E_t-E_{t+1}, V[1]=O_t+E_{t+1}, V[2]=E_{t+1}-O_t, V[3]=O_t-O_{t+1}
#     GEMM per 16-row strip: M[m] = sum_{i,ct} u1[.,i,m,.]^T V[.,h+i,m,.]
#     (24 matmuls of free 512, bf16, 4 PSUM banks), then output combine
#       y_even = M0 + (M1+M2)/2,  y_odd = (M1-M2)/2 - M3
#     on vector/gpsimd, writing the interleaved output tile directly.
#   PE work drops to 2/3 of direct conv (6 instead of 9 multiplies per
#   output pair along the W axis).
import numpy as np

B, T, R, ALPHA = 32, 4, 8, 16
CIN, COUT, K = 256, 256, 3
H = W_SP = 64
SCALING = ALPHA / R
NCORES = 8
S = B // NCORES      # samples per core
NR = T * R * K       # 96 lora rows (padded to 128 partitions)
P = 128
HP = H + 2           # 66 padded
HH = 34              # padded-row half-tile height (rows 0:34 and 32:66)

_CACHE = {}


def _build_nc():
    import concourse.bacc as bacc
    import concourse.mybir as mybir
    import concourse.tile as tile

    f32 = mybir.dt.float32
    f32r = mybir.dt.float32r
    bf16 = mybir.dt.bfloat16
    ALU = mybir.AluOpType

    nc = bacc.Bacc("TRN2", target_bir_lowering=False, debug=False, num_devices=NCORES)

    # x padded + column-deinterleaved (even/odd) in bf16
    xq = nc.declare_dram_parameter("xp", [S, 2, P, HP, 2, 33], bf16, isOutput=False)
    # base weights and LoRA A factor pre-transformed by the Winograd G
    # matrix on host: the (i,m) index replaces the 9 conv taps.
    wt = nc.declare_dram_parameter("wt", [P, 3, 4, 2, COUT], bf16, isOutput=False)
    a3 = nc.declare_dram_parameter("a3", [P, 12, 3, CIN], bf16, isOutput=False)
    b3 = nc.declare_dram_parameter("b3", [P, 3, COUT], f32, isOutput=False)
    alph = nc.declare_dram_parameter("alph", [P, S], f32, isOutput=False)
    outd = nc.declare_dram_parameter("out", [S, 2, P, H, W_SP], f32, isOutput=True)

    HSMP = S // 2  # samples per 512-wide delta-matmul half

    with tile.TileContext(nc) as tc:
        with tc.tile_pool(name="u1_pool", bufs=S) as u1_pool, \
             tc.tile_pool(name="xq_pool", bufs=4) as xq_pool, \
             tc.tile_pool(name="v1_pool", bufs=5) as v1_pool:
            # per-sample Winograd-transformed weights, c-major:
            # u1[c, ct, i, m, o] bf16
            u1s = [u1_pool.tile([P, 2, 3, 4, COUT], bf16, name="u1")
                   for _ in range(S)]
            # V1 quarters: [c, 34 padded rows, m, tj] bf16
            v1q = {}   # (smp, ct, half) -> tile, kept for current+next stage
            xqt = {}

            def issue_xq(smp, ct, half):
                t = xq_pool.tile([P, HH, 2, 33], bf16, name="xq")
                nc.sync.dma_start(
                    t[:, :, :, :], xq[smp, ct, :, 32 * half:32 * half + HH, :, :])
                xqt[(smp, ct, half)] = t

            def transform(smp, ct, half, split=False):
                # gpsimd cannot read PSUM, so it gets the (all-SBUF) input
                # transforms while the vector engine owns the PSUM-reading
                # output combines (keeping them off each other's queues).
                # split=True spreads the 4 ops across both engines (used at
                # stage 0 where transform latency gates the conv start).
                xt = xqt.pop((smp, ct, half))
                v = v1_pool.tile([P, HH, 4, 32], bf16, name="v1")
                E0 = xt[:, :, 0, 0:32]
                E1 = xt[:, :, 0, 1:33]
                O0 = xt[:, :, 1, 0:32]
                O1 = xt[:, :, 1, 1:33]
                e0 = nc.vector if split else nc.gpsimd
                e0.tensor_sub(v[:, :, 0, :], E0, E1)
                e0.tensor_add(v[:, :, 1, :], O0, E1)
                nc.gpsimd.tensor_sub(v[:, :, 2, :], E1, O0)
                nc.gpsimd.tensor_sub(v[:, :, 3, :], O0, O1)
                v1q[(smp, ct, half)] = v

            # ---- phase 1: Winograd-transformed weights via LoRA matmuls ----
            # a3/wt carry the G transform from host, so the GEMM evictions
            # land directly in the final u1 (i,m) slots. For each ct block
            # only s in {ct, ct+1} rows of a3 are nonzero (the c*9+d flat-
            # index decomposition), so the third s-matmul is skipped.
            with tc.tile_pool(name="cst", bufs=1) as cst, \
                 tc.tile_pool(name="dps", bufs=4, space="PSUM") as dps:
                a3_sb = cst.tile([P, 12, 3, CIN], bf16)
                b3_sb = cst.tile([P, 3, COUT], f32)
                alph_sb = cst.tile([P, S], f32)
                wt_sb = cst.tile([P, 3, 4, 2, COUT], bf16)
                b3s_lo = cst.tile([P, 3, S // 2, COUT], bf16)
                b3s_hi = cst.tile([P, 3, S // 2, COUT], bf16)

                # DMA completes strictly in issue order. The delta GEMM
                # consumes a3[dm] every ~2us and the evictions need wt[dm]
                # shortly after, so issue those in fine-grained alternating
                # chunks; the sample-0 x quarters follow.
                nc.sync.dma_start(b3_sb[:, :, :], b3[:, :, :])
                nc.sync.dma_start(alph_sb[:, :], alph[:, :])
                nc.sync.dma_start(a3_sb[:, 0:2], a3[:, 0:2])
                nc.sync.dma_start(wt_sb[:, 0:1], wt[:, 0:1])
                nc.sync.dma_start(a3_sb[:, 2:6], a3[:, 2:6])
                nc.sync.dma_start(wt_sb[:, 1:2], wt[:, 1:2])
                nc.sync.dma_start(a3_sb[:, 6:12], a3[:, 6:12])
                nc.sync.dma_start(wt_sb[:, 2:3], wt[:, 2:3])
                issue_xq(0, 0, 0)
                issue_xq(0, 1, 0)
                issue_xq(0, 0, 1)
                issue_xq(0, 1, 1)

                # alphas*scaling on vector; the per-sample b3 scaling runs
                # on the scalar engine (activation Copy with per-partition
                # scale AP), keeping the vector queue free for evictions.
                nc.vector.tensor_scalar_mul(alph_sb[:, :], alph_sb[:, :],
                                            float(SCALING))
                Copy = mybir.ActivationFunctionType.Copy
                for smp in range(S):
                    dst = b3s_lo if smp < HSMP else b3s_hi
                    for s in range(3):
                        nc.scalar.activation(
                            dst[:, s, smp % HSMP, :], b3_sb[:, s, :], Copy,
                            bias=0.0, scale=alph_sb[:, smp:smp + 1])

                for dm in range(12):
                    i, m = divmod(dm, 4)
                    for half in range(2):
                        # dp holds both ct blocks for one sample-half:
                        # [ct, hsmp, o] (2 banks; bufs=4 keeps the PE 4 deep)
                        dp = dps.tile([P, 2, HSMP, COUT], f32, name="dp")
                        b3sh = b3s_lo if half == 0 else b3s_hi
                        for ct in range(2):
                            for s in (ct, ct + 1):
                                nc.tensor.matmul(
                                    dp[:, ct, :, :],
                                    a3_sb[:, dm, s, ct * P:(ct + 1) * P],
                                    b3sh[:, s, :, :],
                                    start=(s == ct), stop=(s == ct + 1))
                        # one eviction per sample covering both ct blocks
                        for hsmp in range(HSMP):
                            nc.vector.tensor_add(
                                u1s[half * HSMP + hsmp][:, :, i, m, :],
                                dp[:, :, hsmp, :],
                                wt_sb[:, i, m, :, :])

            # stage-0 input transforms, emitted before phase 2 so they sit
            # at the front of the (otherwise empty) engine queues
            transform(0, 0, 0)
            transform(0, 1, 0)

            # ---- phase 2: 1D-Winograd conv ----
            with tc.tile_pool(name="ob_pool", bufs=4) as ob_pool, \
                 tc.tile_pool(name="tt_pool", bufs=6) as tt_pool, \
                 tc.tile_pool(name="cps", bufs=2, space="PSUM") as cps:
                stages = [(smp, half) for smp in range(S) for half in range(2)]
                strip_no = 0
                for k, (smp, half) in enumerate(stages):
                    # prefetch + transform the next stage's quarters
                    if k + 1 < len(stages):
                        nsmp, nhalf = stages[k + 1]
                        if nsmp != 0:
                            issue_xq(nsmp, 0, nhalf)
                            issue_xq(nsmp, 1, nhalf)
                        transform(nsmp, 0, nhalf)
                        transform(nsmp, 1, nhalf)
                    va = v1q.pop((smp, 0, half))
                    vb = v1q.pop((smp, 1, half))
                    for ot in range(2):
                        # the very last strip is split in two so the final
                        # combine+DMA tail after the last matmul is shorter
                        strips = [(0, 16), (16, 16)]
                        if k == len(stages) - 1 and ot == 1:
                            strips = [(0, 16), (16, 8), (24, 8)]
                        for r0, rows in strips:
                            pst = cps.tile([P, 4, rows, 32], f32, name="pst")
                            for m in range(4):
                                for i in range(3):
                                    for ct in range(2):
                                        nc.tensor.matmul(
                                            pst[:, m, :, :],
                                            u1s[smp][:, ct, i, m,
                                                     ot * P:(ot + 1) * P],
                                            (va if ct == 0 else vb)[
                                                :, r0 + i:r0 + i + rows, m, :],
                                            start=(i == 0 and ct == 0),
                                            stop=(i == 2 and ct == 1))
                            # output combine:
                            #   even = M0 + (M1+M2)/2 ; odd = (M1-M2)/2 - M3
                            # Engine ops may read only ONE psum operand, so
                            # the scalar engine first evacuates M1 to SBUF.
                            eng = nc.vector
                            strip_no += 1
                            ob = ob_pool.tile([P, rows, 32, 2], f32, name="ob")
                            s1 = tt_pool.tile([P, rows, 32], f32, name="s1")
                            t1 = tt_pool.tile([P, rows, 32], f32, name="tt")
                            t2 = tt_pool.tile([P, rows, 32], f32, name="tt")
                            nc.scalar.copy(s1[:, :, :], pst[:, 1, :, :])
                            eng.tensor_add(t1[:, :, :], pst[:, 2, :, :],
                                           s1[:, :, :])
                            eng.scalar_tensor_tensor(
                                ob[:, :, :, 0], t1[:, :, :], 0.5,
                                pst[:, 0, :, :], ALU.mult, ALU.add)
                            eng.scalar_tensor_tensor(
                                t2[:, :, :], pst[:, 2, :, :], -1.0,
                                s1[:, :, :], ALU.mult, ALU.add)
                            eng.scalar_tensor_tensor(
                                ob[:, :, :, 1], t2[:, :, :], 0.5,
                                pst[:, 3, :, :], ALU.mult, ALU.subtract)
                            nc.sync.dma_start(
                                outd[smp, ot, :,
                                     32 * half + r0:32 * half + r0 + rows, :],
                                ob[:, :, :, :])
    nc.finalize()
    return nc


def _host_prep(x, alphas, W, lora_A, lora_B):
    """Host-side layout transforms (pad/transpose/gather/deinterleave)."""
    import ml_dtypes
    bf16 = ml_dtypes.bfloat16

    xf = np.ascontiguousarray(np.asarray(x, dtype=np.float32))
    af = np.asarray(alphas, dtype=np.float32)
    Wf = np.asarray(W, dtype=np.float32)
    Af = np.asarray(lora_A, dtype=np.float32).reshape(NR, CIN * K)   # Acat
    Bf = np.asarray(lora_B, dtype=np.float32)

    # padded x, bf16, even/odd columns deinterleaved:
    # (S, 2, 128, 66, 2, 33) per core
    xpad = np.zeros((B, CIN, HP, HP), np.float32)
    xpad[:, :, 1:-1, 1:-1] = xf
    xde = np.empty((B, CIN, HP, 2, 33), bf16)
    xde[:, :, :, 0, :] = xpad[:, :, :, 0::2]
    xde[:, :, :, 1, :] = xpad[:, :, :, 1::2]
    xde = xde.reshape(NCORES, S, 2, P, HP, 2, 33)

    # Winograd F(2,3) weight transform over the w-taps (unscaled G; the
    # 1/2 is folded into the device-side output combine)
    G = np.array([[1, 0, 0], [1, 1, 1], [1, -1, 1], [0, 0, 1]], np.float32)

    # base weights c-major: wt[p, i, m, ct, o] = G-combo of [p, (i,j), ct, o]
    wt9 = (Wf.reshape(COUT, CIN, 9).transpose(1, 2, 0)     # [c, d, o]
           .reshape(2, P, 3, 3, COUT)                      # [ct, p, i, j, o]
           .transpose(1, 2, 3, 0, 4))                      # [p, i, j, ct, o]
    wth = np.ascontiguousarray(
        np.einsum('mj,pijto->pimto', G, wt9)).astype(bf16)

    # a3[r, d, s, c] = Acat[r, c*9+d-768*s] masked; rows padded 96 -> 128,
    # then G-folded over the j tap: a3m[r, (i,m), s, c]
    a3h = np.zeros((P, 9, 3, CIN), np.float32)
    cc = np.arange(CIN)
    for d in range(9):
        q = cc * 9 + d
        s_of_c = q // (CIN * K)
        q_of_c = q % (CIN * K)
        for s in range(3):
            m = s_of_c == s
            a3h[:NR, d, s, m] = Af[:, q_of_c[m]]
    a3h = np.ascontiguousarray(
        np.einsum('mj,rijsc->rimsc', G, a3h.reshape(P, 3, 3, 3, CIN))
        .reshape(P, 12, 3, CIN)).astype(bf16)

    # b3[r, s, o] = Bcat[3o+s, r];  Bcat = lora_B transposed to [768, 96]
    Bcat = Bf.transpose(1, 0, 2).reshape(COUT * K, NR)
    b3h = np.zeros((P, 3, COUT), np.float32)
    b3h[:NR] = Bcat.reshape(COUT, 3, NR).transpose(2, 1, 0)

    # alph[r, smp] per core (repeat each task 24x; zero rows >= 96)
    alphh = np.zeros((NCORES, P, S), np.float32)
    rep = np.repeat(af, R * K, axis=1)                     # [B, 96]
    alphh[:, :NR, :] = rep.reshape(NCORES, S, NR).transpose(0, 2, 1)

    return xde, wth, a3h, b3h, alphh


def kernel(x, alphas, W, lora_A, lora_B):
    from concourse.bass_utils import run_bass_kernel_spmd

    if "nc" not in _CACHE:
        _CACHE["nc"] = _build_nc()
    nc = _CACHE["nc"]

    xpad, wth, a3h, b3h, alphh = _host_prep(x, alphas, W, lora_A, lora_B)
    in_maps = [
        {"xp": np.ascontiguousarray(xpad[c]), "wt": wth, "a3": a3h, "b3": b3h,
         "alph": np.ascontiguousarray(alphh[c])}
        for c in range(NCORES)
    ]
    res = run_bass_kernel_spmd(nc, in_maps, list(range(NCORES)))
    out = np.empty((B, COUT, H, W_SP), np.float32)
    for c in range(NCORES):
        out[c * S:(c + 1) * S] = res.results[c]["out"].reshape(S, COUT, H, W_SP)
    return out


# revision 33
# speedup vs baseline: 1.0649x; 1.0085x over previous
# MultiLoraConv2d kernel for 8 trn2 NeuronCores (Bass/Tile, data-parallel over batch).
#
# Math (per sample b):
#   delta_flat[b] = sum_t 2*alphas[b,t] * (lora_B[t] @ lora_A[t])        [768, 768]
#   agg[b] = W + delta_flat[b].reshape(COUT, CIN, 3, 3)                  (flat reinterpret)
#   out[b] = conv2d(x[b], agg[b], pad=1)
#
# Device strategy (per core, S = B/8 samples):
#   - Phase 1 (unchanged from direct-conv version): per-sample aggregated
#     conv weights come out of the PE in c-major layout via LoRA matmuls;
#     evictions land in w-tap slots of u1 tiles, then 3 vector ops per
#     (sample, cin-tile) build the 1D-Winograd F(2,3) weight transform
#       m0 = g0, m1 = g0+g1+g2, m2 = g0-g1+g2, m3 = g2  (unscaled; the
#     1/2 is folded into the output combine).
#   - Phase 2: conv via 1D Winograd along W. Input transform (4 tensor
#     ops per x quarter, bf16, host-deinterleaved even/odd columns):
#       V[0]=E_t-E_{t+1}, V[1]=O_t+E_{t+1}, V[2]=E_{t+1}-O_t, V[3]=O_t-O_{t+1}
#     GEMM per 16-row strip: M[m] = sum_{i,ct} u1[.,i,m,.]^T V[.,h+i,m,.]
#     (24 matmuls of free 512, bf16, 4 PSUM banks), then output combine
#       y_even = M0 + (M1+M2)/2,  y_odd = (M1-M2)/2 - M3
#     on vector/gpsimd, writing the interleaved output tile directly.
#   PE work drops to 2/3 of direct conv (6 instead of 9 multiplies per
#   output pair along the W axis).
import numpy as np

B, T, R, ALPHA = 32, 4, 8, 16
CIN, COUT, K = 256, 256, 3
H = W_SP = 64
SCALING = ALPHA / R
NCORES = 8
S = B // NCORES      # samples per core
NR = T * R * K       # 96 lora rows (padded to 128 partitions)
P = 128
HP = H + 2           # 66 padded
HH = 34              # padded-row half-tile height (rows 0:34 and 32:66)

_CACHE = {}


def _build_nc():
    import concourse.bacc as bacc
    import concourse.mybir as mybir
    import concourse.tile as tile

    f32 = mybir.dt.float32
    f32r = mybir.dt.float32r
    bf16 = mybir.dt.bfloat16
    ALU = mybir.AluOpType

    nc = bacc.Bacc("TRN2", target_bir_lowering=False, debug=False, num_devices=NCORES)

    # x padded + column-deinterleaved (even/odd) in bf16
    xq = nc.declare_dram_parameter("xp", [S, 2, P, HP, 2, 33], bf16, isOutput=False)
    # base weights and LoRA A factor pre-transformed by the Winograd G
    # matrix on host: the (i,m) index replaces the 9 conv taps.
    wt = nc.declare_dram_parameter("wt", [P, 3, 4, 2, COUT], bf16, isOutput=False)
    a3 = nc.declare_dram_parameter("a3", [P, 12, 3, CIN], bf16, isOutput=False)
    b3 = nc.declare_dram_parameter("b3", [P, 3, COUT], f32, isOutput=False)
    alph = nc.declare_dram_parameter("alph", [P, S], f32, isOutput=False)
    outd = nc.declare_dram_parameter("out", [S, 2, P, H, W_SP], f32, isOutput=True)

    HSMP = S // 2  # samples per 512-wide delta-matmul half

    with tile.TileContext(nc) as tc:
        with tc.tile_pool(name="u1_pool", bufs=S) as u1_pool, \
             tc.tile_pool(name="xq_pool", bufs=4) as xq_pool, \
             tc.tile_pool(name="v1_pool", bufs=5) as v1_pool:
            # per-sample Winograd-transformed weights, c-major:
            # u1[c, ct, i, m, o] bf16
            u1s = [u1_pool.tile([P, 2, 3, 4, COUT], bf16, name="u1")
                   for _ in range(S)]
            # V1 quarters: [c, 34 padded rows, m, tj] bf16
            v1q = {}   # (smp, ct, half) -> tile, kept for current+next stage
            xqt = {}

            def issue_xq(smp, ct, half):
                t = xq_pool.tile([P, HH, 2, 33], bf16, name="xq")
                nc.sync.dma_start(
                    t[:, :, :, :], xq[smp, ct, :, 32 * half:32 * half + HH, :, :])
                xqt[(smp, ct, half)] = t

            def transform(smp, ct, half, split=False):
                # gpsimd cannot read PSUM, so it gets the (all-SBUF) input
                # transforms while the vector engine owns the PSUM-reading
                # output combines (keeping them off each other's queues).
                # split=True spreads the 4 ops across both engines (used at
                # stage 0 where transform latency gates the conv start).
                xt = xqt.pop((smp, ct, half))
                v = v1_pool.tile([P, HH, 4, 32], bf16, name="v1")
                E0 = xt[:, :, 0, 0:32]
                E1 = xt[:, :, 0, 1:33]
                O0 = xt[:, :, 1, 0:32]
                O1 = xt[:, :, 1, 1:33]
                e0 = nc.vector if split else nc.gpsimd
                e0.tensor_sub(v[:, :, 0, :], E0, E1)
                e0.tensor_add(v[:, :, 1, :], O0, E1)
                nc.gpsimd.tensor_sub(v[:, :, 2, :], E1, O0)
                nc.gpsimd.tensor_sub(v[:, :, 3, :], O0, O1)
                v1q[(smp, ct, half)] = v

            # ---- phase 1: Winograd-transformed weights via LoRA matmuls ----
            # a3/wt carry the G transform from host, so the GEMM evictions
            # land directly in the final u1 (i,m) slots. For each ct block
            # only s in {ct, ct+1} rows of a3 are nonzero (the c*9+d flat-
            # index decomposition), so the third s-matmul is skipped.
            with tc.tile_pool(name="cst", bufs=1) as cst, \
                 tc.tile_pool(name="dps", bufs=4, space="PSUM") as dps:
                a3_sb = cst.tile([P, 12, 3, CIN], bf16)
                b3_sb = cst.tile([P, 3, COUT], f32)
                alph_sb = cst.tile([P, S], f32)
                wt_sb = cst.tile([P, 3, 4, 2, COUT], bf16)
                b3s_lo = cst.tile([P, 3, S // 2, COUT], bf16)
                b3s_hi = cst.tile([P, 3, S // 2, COUT], bf16)

                # DMA completes strictly in issue order. The delta GEMM
                # consumes a3[dm] every ~2us and the evictions need wt[dm]
                # shortly after, so issue those in fine-grained alternating
                # chunks; the sample-0 x quarters follow.
                nc.sync.dma_start(alph_sb[:, :], alph[:, :])
                nc.sync.dma_start(b3_sb[:, :, :], b3[:, :, :])
                nc.sync.dma_start(a3_sb[:, 0:2], a3[:, 0:2])
                issue_xq(0, 0, 0)
                issue_xq(0, 1, 0)
                nc.sync.dma_start(wt_sb[:, 0:1], wt[:, 0:1])
                nc.sync.dma_start(a3_sb[:, 2:6], a3[:, 2:6])
                nc.sync.dma_start(wt_sb[:, 1:2], wt[:, 1:2])
                nc.sync.dma_start(a3_sb[:, 6:12], a3[:, 6:12])
                nc.sync.dma_start(wt_sb[:, 2:3], wt[:, 2:3])
                issue_xq(0, 0, 1)
                issue_xq(0, 1, 1)

                # alphas*scaling on vector; the per-sample b3 scaling runs
                # on the scalar engine (activation Copy with per-partition
                # scale AP), keeping the vector queue free for evictions.
                nc.vector.tensor_scalar_mul(alph_sb[:, :], alph_sb[:, :],
                                            float(SCALING))
                # s-major so the first delta matmuls' inputs complete first
                Copy = mybir.ActivationFunctionType.Copy
                for s in range(3):
                    for smp in range(S):
                        dst = b3s_lo if smp < HSMP else b3s_hi
                        nc.scalar.activation(
                            dst[:, s, smp % HSMP, :], b3_sb[:, s, :], Copy,
                            bias=0.0, scale=alph_sb[:, smp:smp + 1])

                for dm in range(12):
                    i, m = divmod(dm, 4)
                    for half in range(2):
                        # dp holds both ct blocks for one sample-half:
                        # [ct, hsmp, o] (2 banks; bufs=4 keeps the PE 4 deep)
                        dp = dps.tile([P, 2, HSMP, COUT], f32, name="dp")
                        b3sh = b3s_lo if half == 0 else b3s_hi
                        for ct in range(2):
                            for s in (ct, ct + 1):
                                nc.tensor.matmul(
                                    dp[:, ct, :, :],
                                    a3_sb[:, dm, s, ct * P:(ct + 1) * P],
                                    b3sh[:, s, :, :],
                                    start=(s == ct), stop=(s == ct + 1))
                        # one eviction per sample covering both ct blocks
                        for hsmp in range(HSMP):
                            nc.vector.tensor_add(
                                u1s[half * HSMP + hsmp][:, :, i, m, :],
                                dp[:, :, hsmp, :],
                                wt_sb[:, i, m, :, :])

            # stage-0 input transforms, emitted before phase 2 so they sit
            # at the front of the (otherwise empty) engine queues
            transform(0, 0, 0)
            transform(0, 1, 0)

            # ---- phase 2: 1D-Winograd conv ----
            with tc.tile_pool(name="ob_pool", bufs=4) as ob_pool, \
                 tc.tile_pool(name="tt_pool", bufs=6) as tt_pool, \
                 tc.tile_pool(name="cps", bufs=2, space="PSUM") as cps:
                stages = [(smp, half) for smp in range(S) for half in range(2)]
                strip_no = 0
                for k, (smp, half) in enumerate(stages):
                    # prefetch + transform the next stage's quarters
                    if k + 1 < len(stages):
                        nsmp, nhalf = stages[k + 1]
                        if nsmp != 0:
                            issue_xq(nsmp, 0, nhalf)
                            issue_xq(nsmp, 1, nhalf)
                        transform(nsmp, 0, nhalf)
                        transform(nsmp, 1, nhalf)
                    va = v1q.pop((smp, 0, half))
                    vb = v1q.pop((smp, 1, half))
                    for ot in range(2):
                        # the very last strip is split in two so the final
                        # combine+DMA tail after the last matmul is shorter
                        strips = [(0, 16), (16, 16)]
                        if k == len(stages) - 1 and ot == 1:
                            strips = [(0, 16), (16, 8), (24, 8)]
                        for r0, rows in strips:
                            pst = cps.tile([P, 4, rows, 32], f32, name="pst")
                            for m in range(4):
                                for i in range(3):
                                    for ct in range(2):
                                        nc.tensor.matmul(
                                            pst[:, m, :, :],
                                            u1s[smp][:, ct, i, m,
                                                     ot * P:(ot + 1) * P],
                                            (va if ct == 0 else vb)[
                                                :, r0 + i:r0 + i + rows, m, :],
                                            start=(i == 0 and ct == 0),
                                            stop=(i == 2 and ct == 1))
                            # output combine:
                            #   even = M0 + (M1+M2)/2 ; odd = (M1-M2)/2 - M3
                            # Engine ops may read only ONE psum operand, so
                            # the scalar engine first evacuates M1 to SBUF.
                            eng = nc.vector
                            strip_no += 1
                            ob = ob_pool.tile([P, rows, 32, 2], f32, name="ob")
                            s1 = tt_pool.tile([P, rows, 32], f32, name="s1")
                            t1 = tt_pool.tile([P, rows, 32], f32, name="tt")
                            t2 = tt_pool.tile([P, rows, 32], f32, name="tt")
                            nc.scalar.copy(s1[:, :, :], pst[:, 1, :, :])
                            eng.tensor_add(t1[:, :, :], pst[:, 2, :, :],
                                           s1[:, :, :])
                            eng.scalar_tensor_tensor(
                                ob[:, :, :, 0], t1[:, :, :], 0.5,
                                pst[:, 0, :, :], ALU.mult, ALU.add)
                            eng.scalar_tensor_tensor(
                                t2[:, :, :], pst[:, 2, :, :], -1.0,
                                s1[:, :, :], ALU.mult, ALU.add)
                            eng.scalar_tensor_tensor(
                                ob[:, :, :, 1], t2[:, :, :], 0.5,
                                pst[:, 3, :, :], ALU.mult, ALU.subtract)
                            nc.sync.dma_start(
                                outd[smp, ot, :,
                                     32 * half + r0:32 * half + r0 + rows, :],
                                ob[:, :, :, :])
    nc.finalize()
    return nc


def _host_prep(x, alphas, W, lora_A, lora_B):
    """Host-side layout transforms (pad/transpose/gather/deinterleave)."""
    import ml_dtypes
    bf16 = ml_dtypes.bfloat16

    xf = np.ascontiguousarray(np.asarray(x, dtype=np.float32))
    af = np.asarray(alphas, dtype=np.float32)
    Wf = np.asarray(W, dtype=np.float32)
    Af = np.asarray(lora_A, dtype=np.float32).reshape(NR, CIN * K)   # Acat
    Bf = np.asarray(lora_B, dtype=np.float32)

    # padded x, bf16, even/odd columns deinterleaved:
    # (S, 2, 128, 66, 2, 33) per core
    xpad = np.zeros((B, CIN, HP, HP), np.float32)
    xpad[:, :, 1:-1, 1:-1] = xf
    xde = np.empty((B, CIN, HP, 2, 33), bf16)
    xde[:, :, :, 0, :] = xpad[:, :, :, 0::2]
    xde[:, :, :, 1, :] = xpad[:, :, :, 1::2]
    xde = xde.reshape(NCORES, S, 2, P, HP, 2, 33)

    # Winograd F(2,3) weight transform over the w-taps (unscaled G; the
    # 1/2 is folded into the device-side output combine)
    G = np.array([[1, 0, 0], [1, 1, 1], [1, -1, 1], [0, 0, 1]], np.float32)

    # base weights c-major: wt[p, i, m, ct, o] = G-combo of [p, (i,j), ct, o]
    wt9 = (Wf.reshape(COUT, CIN, 9).transpose(1, 2, 0)     # [c, d, o]
           .reshape(2, P, 3, 3, COUT)                      # [ct, p, i, j, o]
           .transpose(1, 2, 3, 0, 4))                      # [p, i, j, ct, o]
    wth = np.ascontiguousarray(
        np.einsum('mj,pijto->pimto', G, wt9)).astype(bf16)

    # a3[r, d, s, c] = Acat[r, c*9+d-768*s] masked; rows padded 96 -> 128,
    # then G-folded over the j tap: a3m[r, (i,m), s, c]
    a3h = np.zeros((P, 9, 3, CIN), np.float32)
    cc = np.arange(CIN)
    for d in range(9):
        q = cc * 9 + d
        s_of_c = q // (CIN * K)
        q_of_c = q % (CIN * K)
        for s in range(3):
            m = s_of_c == s
            a3h[:NR, d, s, m] = Af[:, q_of_c[m]]
    a3h = np.ascontiguousarray(
        np.einsum('mj,rijsc->rimsc', G, a3h.reshape(P, 3, 3, 3, CIN))
        .reshape(P, 12, 3, CIN)).astype(bf16)

    # b3[r, s, o] = Bcat[3o+s, r];  Bcat = lora_B transposed to [768, 96]
    Bcat = Bf.transpose(1, 0, 2).reshape(COUT * K, NR)
    b3h = np.zeros((P, 3, COUT), np.float32)
    b3h[:NR] = Bcat.reshape(COUT, 3, NR).transpose(2, 1, 0)

    # alph[r, smp] per core (repeat each task 24x; zero rows >= 96)
    alphh = np.zeros((NCORES, P, S), np.float32)
    rep = np.repeat(af, R * K, axis=1)                     # [B, 96]
    alphh[:, :NR, :] = rep.reshape(NCORES, S, NR).transpose(0, 2, 1)

    return xde, wth, a3h, b3h, alphh


def kernel(x, alphas, W, lora_A, lora_B):
    from concourse.bass_utils import run_bass_kernel_spmd

    if "nc" not in _CACHE:
        _CACHE["nc"] = _build_nc()
    nc = _CACHE["nc"]

    xpad, wth, a3h, b3h, alphh = _host_prep(x, alphas, W, lora_A, lora_B)
    in_maps = [
        {"xp": np.ascontiguousarray(xpad[c]), "wt": wth, "a3": a3h, "b3": b3h,
         "alph": np.ascontiguousarray(alphh[c])}
        for c in range(NCORES)
    ]
    res = run_bass_kernel_spmd(nc, in_maps, list(range(NCORES)))
    out = np.empty((B, COUT, H, W_SP), np.float32)
    for c in range(NCORES):
        out[c * S:(c + 1) * S] = res.results[c]["out"].reshape(S, COUT, H, W_SP)
    return out
